# revision 3
# baseline (speedup 1.0000x reference)
"""Multi-head dot-product attention (causal, f32) on 8 TRN2 NeuronCores.

Sharding (Megatron-style, per sharding hint): batch (2) x head-groups (4 of
4 heads) = 8 cores. Each core computes q/k/v projections for its 4 heads,
causal attention, and the partial output projection Y_c = sum_h O_h @ Wo_h
for its batch. Host sums the 4 partial Y per batch (the "all-reduce").

Kernel layout strategy: all activations live in transposed [feature, token]
layout so every matmul contracts over the partition dim with N=512 moving
operands:
    QT_h[d,t]  = sum_e Wq[e,hd] * XqT[e,t]      (stationary Wq chunk)
    KT_h[d,s]  = sum_e Wk[e,hd] * XkvT[e,s]
    V[s,hd]    = sum_e XkvT[e,s-blk] * Wv[e,:]  (stationary XkvT chunk)
    LT[s,t]    = KT_h[:,s-blk].T @ QT_h[:,t]    (one matmul, K=D=128)
    P = exp(scale*(LT + causal_bias))           (ACT, writes f32r)
    OT_h[d,t] += V[s-blk,hd].T @ P              (accumulate over s blocks)
    R[1,t]    += ones.T @ P                     (softmax denominator)
    OT_h     *= broadcast(1/R)                  (K=1 outer-product matmul)
    Y[t,e]    = sum_h OT_h[:,t-blk].T @ Wo[h][:,e]
Causal masking skips fully-masked s-blocks and adds -1e10 bias on diagonal
blocks before the exp. Matmuls run in f32r (TF32-class) at full PE rate.
"""
import math
import os
import numpy as np

import concourse.bass as bass
import concourse.mybir as mybir
import concourse.tile as tile
from concourse import bacc
from concourse import bass_utils

f32 = mybir.dt.float32
f32r = mybir.dt.float32r
AF = mybir.ActivationFunctionType

# Problem shape (hardcoded per contract)
B, T, S, E, N, D = 2, 2048, 2048, 2048, 16, 128
N_CORES = 8
HL = 4            # heads per core
P = 128           # partitions


def build_nc(T=T, S=S, E=E, HL=HL, TT=512, ST=512):
    """Build the single-core SPMD bass program."""
    NE = E // P           # contraction chunks for projections
    NTT = T // TT         # t tiles
    NST = S // ST         # s tiles in kv phase
    NSB_PER_ST = ST // P  # s blocks per s tile
    NDIAG = TT // P       # diagonal mask patterns
    SCALE = 1.0 / math.sqrt(D)

    nc = bacc.Bacc("TRN2", target_bir_lowering=False, debug=False)
    xqT = nc.dram_tensor("xqT", [E, T], f32r, kind="ExternalInput")
    xkvT = nc.dram_tensor("xkvT", [E, S], f32r, kind="ExternalInput")
    wq = nc.dram_tensor("wq", [E, HL * D], f32r, kind="ExternalInput")
    wk = nc.dram_tensor("wk", [E, HL * D], f32r, kind="ExternalInput")
    wv = nc.dram_tensor("wv", [E, HL * D], f32r, kind="ExternalInput")
    wo = nc.dram_tensor("wo", [HL, D, E], f32r, kind="ExternalInput")
    y = nc.dram_tensor("y", [T, E], f32, kind="ExternalOutput")

    with tile.TileContext(nc) as tc:
        with tc.tile_pool(name="persist", bufs=1) as persist:
            kt_all = persist.tile([P, HL, S], f32r)         # K^T [d, h, s]
            v_all = persist.tile([P, S // P, HL * D], f32r)  # V [s-part, blk, hd]
            mbp = persist.tile([P, NDIAG, TT], f32)          # diag causal bias
            ones_row = persist.tile([1, P], f32r)            # [1,128] lhsT for K=1 bcast
            ones_col = persist.tile([P, 1], f32r)            # [128,1] lhsT for rowsum
            scr = persist.tile([P, P], f32)

            # constants
            nc.gpsimd.memset(scr[:], 1.0)
            nc.vector.tensor_copy(ones_row[:], scr[0:1, :])
            nc.vector.tensor_copy(ones_col[:], scr[:, 0:1])
            for k in range(NDIAG):
                nc.gpsimd.memset(mbp[:, k, :], 0.0)
                # keep 0 where (t_in_tile) - (s_in_blk) - 128k >= 0, else -1e10
                nc.gpsimd.affine_select(
                    out=mbp[:, k, :], in_=mbp[:, k, :],
                    compare_op=mybir.AluOpType.is_ge,
                    fill=-1e10, base=-P * k,
                    pattern=[[1, TT]], channel_multiplier=-1,
                )

            # ---------------- Phase B: K^T and V for all heads ----------------
            with (
                tc.tile_pool(name="wkv", bufs=1) as wkvp,
                tc.tile_pool(name="xkv", bufs=4) as xp,
                tc.tile_pool(name="pskt", bufs=HL, space="PSUM") as pskt,
                tc.tile_pool(name="psv", bufs=NSB_PER_ST, space="PSUM") as psv,
            ):
                wk_t = wkvp.tile([P, NE, HL * D], f32r)
                wv_t = wkvp.tile([P, NE, HL * D], f32r)
                for e in range(NE):
                    nc.sync.dma_start(wk_t[:, e, :], wk[e * P:(e + 1) * P, :])
                    nc.sync.dma_start(wv_t[:, e, :], wv[e * P:(e + 1) * P, :])
                for st in range(NST):
                    psKT = [pskt.tile([P, ST], f32, tag="pskt", name=f"psKT{st}_{h}") for h in range(HL)]
                    psV = [psv.tile([P, HL * D], f32, tag="psv", name=f"psV{st}_{j}") for j in range(NSB_PER_ST)]
                    for e in range(NE):
                        xt = xp.tile([P, ST], f32r, tag="xkv")
                        nc.sync.dma_start(xt[:], xkvT[e * P:(e + 1) * P, st * ST:(st + 1) * ST])
                        for h in range(HL):
                            nc.tensor.matmul(psKT[h][:], wk_t[:, e, h * D:(h + 1) * D],
                                             xt[:], start=(e == 0), stop=(e == NE - 1))
                        for j in range(NSB_PER_ST):
                            nc.tensor.matmul(psV[j][:], xt[:, j * P:(j + 1) * P],
                                             wv_t[:, e, :], start=(e == 0), stop=(e == NE - 1))
                    for h in range(HL):
                        nc.vector.tensor_copy(kt_all[:, h, st * ST:(st + 1) * ST], psKT[h][:])
                    for j in range(NSB_PER_ST):
                        nc.vector.tensor_copy(v_all[:, st * NSB_PER_ST + j, :], psV[j][:])

            # ------------- Phase C: Q, attention, output projection -------------
            with (
                tc.tile_pool(name="wqo", bufs=1) as wqop,
                tc.tile_pool(name="xq", bufs=4) as xqp,
                tc.tile_pool(name="qt", bufs=1) as qtp,
                tc.tile_pool(name="ep", bufs=4) as epp,
                tc.tile_pool(name="on", bufs=1) as onp,
                tc.tile_pool(name="rb", bufs=2) as rbp,
                tc.tile_pool(name="ysb", bufs=4) as yp,
                tc.tile_pool(name="psqy", bufs=HL, space="PSUM") as psqy,
                tc.tile_pool(name="psl", bufs=2, space="PSUM") as psl,
                tc.tile_pool(name="pso", bufs=1, space="PSUM") as pso,
                tc.tile_pool(name="psr", bufs=1, space="PSUM") as psr,
            ):
                wq_t = wqop.tile([P, NE, HL * D], f32r)
                wo_t = wqop.tile([P, HL, E], f32r)
                for e in range(NE):
                    nc.sync.dma_start(wq_t[:, e, :], wq[e * P:(e + 1) * P, :])
                for h in range(HL):
                    nc.sync.dma_start(wo_t[:, h, :], wo[h])

                for tt in range(NTT):
                    # Q projection for this t tile
                    psQT = [psqy.tile([P, TT], f32, tag="psqy", name=f"psQT{tt}_{h}") for h in range(HL)]
                    for e in range(NE):
                        xt = xqp.tile([P, TT], f32r, tag="xq")
                        nc.sync.dma_start(xt[:], xqT[e * P:(e + 1) * P, tt * TT:(tt + 1) * TT])
                        for h in range(HL):
                            nc.tensor.matmul(psQT[h][:], wq_t[:, e, h * D:(h + 1) * D],
                                             xt[:], start=(e == 0), stop=(e == NE - 1))
                    qt_sb = qtp.tile([P, HL, TT], f32r, tag="qt")
                    for h in range(HL):
                        nc.scalar.activation(qt_sb[:, h, :], psQT[h][:], AF.Copy)

                    # attention per head
                    onorm = onp.tile([P, HL, TT], f32r, tag="on")
                    nsb = (tt + 1) * (TT // P)  # causal: s blocks 0..nsb-1
                    for h in range(HL):
                        psO = pso.tile([P, TT], f32, tag="pso")
                        psR = psr.tile([1, TT], f32, tag="psr")
                        for sb in range(nsb):
                            psL = psl.tile([P, TT], f32, tag="psl")
                            nc.tensor.matmul(psL[:], kt_all[:, h, sb * P:(sb + 1) * P],
                                             qt_sb[:, h, :])
                            k = sb - tt * (TT // P)
                            if k >= 0:
                                nc.vector.tensor_add(psL[:], psL[:], mbp[:, k, :])
                            ep_t = epp.tile([P, TT], f32r, tag="ep")
                            nc.scalar.activation(ep_t[:], psL[:], AF.Exp, scale=SCALE)
                            nc.tensor.matmul(psO[:], v_all[:, sb, h * D:(h + 1) * D],
                                             ep_t[:], start=(sb == 0), stop=(sb == nsb - 1))
                            nc.tensor.matmul(psR[:], ones_col[:], ep_t[:],
                                             start=(sb == 0), stop=(sb == nsb - 1))
                        # normalize: OT_h * broadcast(1/R)
                        rb_f = rbp.tile([P, TT], f32, tag="rbf")
                        nc.vector.reciprocal(rb_f[0:1, :], psR[:])
                        rb_r = rbp.tile([P, TT], f32r, tag="rbr")
                        nc.vector.tensor_copy(rb_r[0:1, :], rb_f[0:1, :])
                        psB = psl.tile([P, TT], f32, tag="psl", name="psB")
                        nc.tensor.matmul(psB[:], ones_row[:], rb_r[0:1, :])
                        rbc = rbp.tile([P, TT], f32, tag="rbc")
                        nc.scalar.activation(rbc[:], psB[:], AF.Copy)
                        nc.vector.tensor_mul(onorm[:, h, :], psO[:], rbc[:])

                    # output projection: Y[t-blk, e-tile] = sum_h OT_h.T @ Wo_h
                    for j in range(TT // P):
                        t0 = tt * TT + j * P
                        for et in range(E // TT):
                            psY = psqy.tile([P, TT], f32, tag="psqy", name="psY")
                            for h in range(HL):
                                nc.tensor.matmul(psY[:], onorm[:, h, j * P:(j + 1) * P],
                                                 wo_t[:, h, et * TT:(et + 1) * TT],
                                                 start=(h == 0), stop=(h == HL - 1))
                            y_t = yp.tile([P, TT], f32, tag="y")
                            nc.vector.tensor_copy(y_t[:], psY[:])
                            nc.sync.dma_start(y[t0:t0 + P, et * TT:(et + 1) * TT], y_t[:])

    nc.compile()
    return nc


_NC_CACHE = {}


def _get_nc(key=(T, S, E, HL)):
    if key not in _NC_CACHE:
        _NC_CACHE[key] = build_nc(T=key[0], S=key[1], E=key[2], HL=key[3])
    return _NC_CACHE[key]


def kernel(inputs_q, inputs_kv, Wq, Wk, Wv, Wo):
    inputs_q = np.asarray(inputs_q, dtype=np.float32)
    inputs_kv = np.asarray(inputs_kv, dtype=np.float32)
    Wq = np.asarray(Wq, dtype=np.float32)
    Wk = np.asarray(Wk, dtype=np.float32)
    Wv = np.asarray(Wv, dtype=np.float32)
    Wo = np.asarray(Wo, dtype=np.float32)

    nc = _get_nc()

    # shard: core c -> batch c//4, heads [ (c%4)*4, +4 )
    xqT_b = [np.ascontiguousarray(inputs_q[b].T) for b in range(B)]
    xkvT_b = [np.ascontiguousarray(inputs_kv[b].T) for b in range(B)]
    in_maps = []
    for c in range(N_CORES):
        b, g = divmod(c, N_CORES // B)
        h0 = g * HL
        in_maps.append({
            "xqT": xqT_b[b],
            "xkvT": xkvT_b[b],
            "wq": np.ascontiguousarray(Wq[:, h0:h0 + HL, :].reshape(E, HL * D)),
            "wk": np.ascontiguousarray(Wk[:, h0:h0 + HL, :].reshape(E, HL * D)),
            "wv": np.ascontiguousarray(Wv[:, h0:h0 + HL, :].reshape(E, HL * D)),
            "wo": np.ascontiguousarray(Wo[h0:h0 + HL]),
        })

    res = bass_utils.run_bass_kernel_spmd(nc, in_maps, core_ids=list(range(N_CORES)))

    out = np.zeros((B, T, E), dtype=np.float32)
    for c in range(N_CORES):
        b = c // (N_CORES // B)
        out[b] += res.results[c]["y"]
    return out


# revision 13
# speedup vs baseline: 1.3145x; 1.3145x over previous
"""Multi-head dot-product attention (causal, f32) on 8 TRN2 NeuronCores.

Sharding (Megatron-style, per sharding hint): batch (2) x head-groups (4 of
4 heads) = 8 cores. Each core computes q/k/v projections for its 4 heads,
causal attention, and the partial output projection Y_c = sum_h O_h @ Wo_h
for its batch. Host sums the 4 partial Y per batch (the "all-reduce").

Kernel layout strategy: all activations live in transposed [feature, token]
layout so every matmul contracts over the partition dim with N=512 moving
operands:
    QT_h[d,t]  = sum_e Wq[e,hd] * XqT[e,t]      (stationary Wq chunk)
    KT_h[d,s]  = sum_e Wk[e,hd] * XkvT[e,s]
    V[s,hd]    = sum_e XkvT[e,s-blk] * Wv[e,:]  (stationary XkvT chunk)
    LT[s,t]    = KT_h[:,s-blk].T @ QT_h[:,t]    (one matmul, K=D=128)
    P = exp(scale*(LT + causal_bias))           (ACT, writes f32r)
    OT_h[d,t] += V[s-blk,hd].T @ P              (accumulate over s blocks)
    R[1,t]    += ones.T @ P                     (softmax denominator)
    OT_h     *= broadcast(1/R)                  (K=1 outer-product matmul)
    Y[t,e]    = sum_h OT_h[:,t-blk].T @ Wo[h][:,e]
Causal masking skips fully-masked s-blocks and adds -1e10 bias on diagonal
blocks before the exp. Matmuls run in f32r (TF32-class) at full PE rate.

DMA queues: activation tiles + weights split across the two HWDGE queues
(nc.sync / nc.scalar) chunk-interleaved so the first matmul of each phase
starts after ~2 chunk loads; y stores go out on the SWDGE (gpsimd) queue.
"""
import math
import numpy as np

import concourse.bass as bass
import concourse.mybir as mybir
import concourse.tile as tile
from concourse import bacc
from concourse import bass_utils
from concourse.masks import make_identity

f32 = mybir.dt.float32
f32r = mybir.dt.float32r
AF = mybir.ActivationFunctionType

# Problem shape (hardcoded per contract)
B, T, S, E, N, D = 2, 2048, 2048, 2048, 16, 128
N_CORES = 8
HL = 4            # heads per core
P = 128           # partitions


MM_LABELS = {}


def build_nc(T=T, S=S, E=E, HL=HL, TT=512, ST=512):
    """Build the single-core SPMD bass program."""
    NE = E // P           # contraction chunks for projections
    NTT = T // TT         # t tiles
    NST = S // ST         # s tiles in kv phase
    NSB_PER_ST = ST // P  # s blocks per s tile
    NDIAG = TT // P       # diagonal mask patterns
    MBW = TT + (NDIAG - 1) * P  # wide causal-bias pattern
    SCALE = 1.0 / math.sqrt(D)

    nc = bacc.Bacc("TRN2", target_bir_lowering=False, debug=False)

    def mm(label, *args, **kw):
        r = nc.tensor.matmul(*args, **kw)
        MM_LABELS[r.ins.name] = label
        return r

    xqT = nc.dram_tensor("xqT", [E, T], f32r, kind="ExternalInput")
    xkvT = nc.dram_tensor("xkvT", [E, S], f32r, kind="ExternalInput")
    wq = nc.dram_tensor("wq", [E, HL * D], f32r, kind="ExternalInput")
    wk = nc.dram_tensor("wk", [E, HL * D], f32r, kind="ExternalInput")
    wv = nc.dram_tensor("wv", [E, HL * D], f32r, kind="ExternalInput")
    wo = nc.dram_tensor("wo", [HL, D, E], f32r, kind="ExternalInput")
    y = nc.dram_tensor("y", [T, E], f32, kind="ExternalOutput")

    with tile.TileContext(nc) as tc:
        with tc.tile_pool(name="persist", bufs=1) as persist:
            kt_all = persist.tile([P, HL, S], f32r)          # K^T [d, h, s]
            v_all = persist.tile([P, S // P, HL * D], f32r)  # V [s-part, blk, hd]
            mbp = persist.tile([P, MBW], f32r)               # wide causal bias (f32r)
            ident = persist.tile([P, P], f32r)               # f32r identity (mask matmul)
            ones_row = persist.tile([1, P], f32r)            # [1,128] lhsT for K=1 bcast
            ones_col = persist.tile([P, 1], f32r)            # [128,1] lhsT for rowsum

            with tc.tile_pool(name="init", bufs=1) as initp:
                scr = initp.tile([P, MBW], f32)
                nc.gpsimd.memset(scr[:, 0:P], 1.0)
                nc.vector.tensor_copy(ones_row[:], scr[0:1, 0:P])
                nc.vector.tensor_copy(ones_col[:], scr[:, 0:1])
                idf = initp.tile([P, P], f32)
                make_identity(nc, idf[:])
                nc.vector.tensor_copy(ident[:], idf[:])
                # wide pattern W[si, u]: 0 where (u - (NDIAG-1)*P) - si >= 0 else -1e10
                # diag pattern k is the slice mbp[:, (NDIAG-1-k)*P : +TT]
                nc.gpsimd.memset(scr[:], 0.0)
                nc.gpsimd.affine_select(
                    out=scr[:], in_=scr[:],
                    compare_op=mybir.AluOpType.is_ge,
                    fill=-1e10, base=-(NDIAG - 1) * P,
                    pattern=[[1, MBW]], channel_multiplier=-1,
                )
                nc.vector.tensor_copy(mbp[:], scr[:])

            def mb_slice(k):
                off = (NDIAG - 1 - k) * P
                return mbp[:, off:off + TT]

            # wq prefetched during phase B (scalar queue, after wk/wv)
            wqp_cm = tc.tile_pool(name="wqp", bufs=1)
            wqp = wqp_cm.__enter__()
            wq_t = wqp.tile([P, NE, HL * D], f32r)

            # ---------------- Phase B: K^T and V for all heads ----------------
            with (
                tc.tile_pool(name="wkv", bufs=1) as wkvp,
                tc.tile_pool(name="xkv", bufs=6) as xp,
                tc.tile_pool(name="pskt", bufs=HL, space="PSUM") as pskt,
                tc.tile_pool(name="psv", bufs=NSB_PER_ST, space="PSUM") as psv,
            ):
                wk_t = wkvp.tile([P, NE, HL * D], f32r)
                wv_t = wkvp.tile([P, NE, HL * D], f32r)
                # weights on the scalar HWDGE queue, chunk-interleaved
                for e in range(NE):
                    nc.scalar.dma_start(wk_t[:, e, :], wk[e * P:(e + 1) * P, :])
                    nc.scalar.dma_start(wv_t[:, e, :], wv[e * P:(e + 1) * P, :])
                for e in range(NE):
                    nc.scalar.dma_start(wq_t[:, e, :], wq[e * P:(e + 1) * P, :])
                for st in range(NST):
                    psKT = [pskt.tile([P, ST], f32, tag="pskt", name=f"psKT{st}_{h}")
                            for h in range(HL)]
                    psV = [psv.tile([P, HL * D], f32, tag="psv", name=f"psV{st}_{j}")
                           for j in range(NSB_PER_ST)]
                    for e in range(NE):
                        xt = xp.tile([P, ST], f32r, tag="xkv")
                        nc.sync.dma_start(xt[:], xkvT[e * P:(e + 1) * P, st * ST:(st + 1) * ST])
                        for h in range(HL):
                            mm("KT", psKT[h][:], wk_t[:, e, h * D:(h + 1) * D],
                               xt[:], start=(e == 0), stop=(e == NE - 1))
                        for j in range(NSB_PER_ST):
                            mm("V", psV[j][:], xt[:, j * P:(j + 1) * P],
                               wv_t[:, e, :], start=(e == 0), stop=(e == NE - 1))
                    for h in range(HL):
                        nc.vector.tensor_copy(kt_all[:, h, st * ST:(st + 1) * ST], psKT[h][:])
                    for j in range(NSB_PER_ST):
                        nc.vector.tensor_copy(v_all[:, st * NSB_PER_ST + j, :], psV[j][:])

            # ------------- Phase C: Q, attention, output projection -------------
            with (
                tc.tile_pool(name="wop", bufs=8) as wop,
                tc.tile_pool(name="xq", bufs=NE + 2) as xqp,
                tc.tile_pool(name="qt", bufs=2) as qtp,
                tc.tile_pool(name="ep", bufs=4) as epp,
                tc.tile_pool(name="on", bufs=2) as onp,
                tc.tile_pool(name="rb", bufs=1) as rbp,
                tc.tile_pool(name="ysb", bufs=4) as yp,
                tc.tile_pool(name="psq", bufs=2, space="PSUM") as psq,
                tc.tile_pool(name="psy", bufs=2, space="PSUM") as psy,
                tc.tile_pool(name="psl", bufs=2, space="PSUM") as psl,
                tc.tile_pool(name="pso", bufs=1, space="PSUM") as pso,
                tc.tile_pool(name="psr", bufs=1, space="PSUM") as psr,
            ):
                def q_proj(tt):
                    # Q projection: cached x chunks, two sweeps of 2 heads
                    qt_sb = qtp.tile([P, HL, TT], f32r, tag="qt", name=f"qt{tt}")
                    xts = []
                    for e in range(NE):
                        xt = xqp.tile([P, TT], f32r, tag="xq", name=f"xq{tt}_{e}")
                        nc.sync.dma_start(xt[:], xqT[e * P:(e + 1) * P, tt * TT:(tt + 1) * TT])
                        xts.append(xt)
                    for sweep in range(HL // 2):
                        hs = (sweep * 2, sweep * 2 + 1)
                        psQT = [psq.tile([P, TT], f32, tag="psq", name=f"psQT{tt}_{h}")
                                for h in hs]
                        for e in range(NE):
                            for i, h in enumerate(hs):
                                mm("Q", psQT[i][:], wq_t[:, e, h * D:(h + 1) * D],
                                   xts[e][:], start=(e == 0), stop=(e == NE - 1))
                        for i, h in enumerate(hs):
                            nc.scalar.activation(qt_sb[:, h, :], psQT[i][:], AF.Copy)
                    return qt_sb

                def attention(tt, qt_sb, after_head=None):
                    onorm = onp.tile([P, HL, TT], f32r, tag="on", name=f"on{tt}")
                    nsb = (tt + 1) * (TT // P)  # causal: s blocks 0..nsb-1
                    diag0 = tt * (TT // P)

                    def qk(h, sb):
                        # diagonal blocks: mask bias enters via PE matmul
                        # (identity x pattern), then QK accumulates on top
                        psL = psl.tile([P, TT], f32, tag="psl", name="psL")
                        k = sb - diag0
                        if k >= 0:
                            mm("MB", psL[:], ident[:], mb_slice(k),
                               start=True, stop=False)
                            mm("QK", psL[:], kt_all[:, h, sb * P:(sb + 1) * P],
                               qt_sb[:, h, :], start=False, stop=True)
                        else:
                            mm("QK", psL[:], kt_all[:, h, sb * P:(sb + 1) * P],
                               qt_sb[:, h, :])
                        return psL

                    for h in range(HL):
                        psO = pso.tile([P, TT], f32, tag="pso", name="psO")
                        psR = psr.tile([1, TT], f32, tag="psr", name="psR")
                        psL_next = qk(h, 0)
                        for sb in range(nsb):
                            psL_cur = psL_next
                            ep_t = epp.tile([P, TT], f32r, tag="ep", name="ep")
                            nc.scalar.activation(ep_t[:], psL_cur[:], AF.Exp, scale=SCALE)
                            if sb + 1 < nsb:
                                psL_next = qk(h, sb + 1)  # PE runs ahead of exp
                            mm("AV", psO[:], v_all[:, sb, h * D:(h + 1) * D],
                               ep_t[:], start=(sb == 0), stop=(sb == nsb - 1))
                            mm("R", psR[:], ones_col[:], ep_t[:],
                               start=(sb == 0), stop=(sb == nsb - 1))
                        # normalize: OT_h * broadcast(1/R)
                        rb_f = rbp.tile([P, TT], f32, tag="rbf")
                        nc.vector.reciprocal(rb_f[0:1, :], psR[:])
                        rb_r = rbp.tile([P, TT], f32r, tag="rbr")
                        nc.vector.tensor_copy(rb_r[0:1, :], rb_f[0:1, :])
                        psB = psr.tile([P, TT], f32, tag="psr", name="psB")
                        mm("BC", psB[:], ones_row[:], rb_r[0:1, :])
                        rbc = rbp.tile([P, TT], f32, tag="rbc")
                        nc.scalar.activation(rbc[:], psB[:], AF.Copy)
                        nc.vector.tensor_mul(onorm[:, h, :], psO[:], rbc[:])
                        if after_head is not None:
                            after_head(h)
                    return onorm

                def out_proj_chunk(tt, onorm, et):
                    # one e-tile of the output projection; wo streamed on the
                    # scalar HWDGE queue
                    wo_et = []
                    for h in range(HL):
                        w = wop.tile([P, TT], f32r, tag="wo", name=f"wo{tt}_{et}_{h}")
                        nc.scalar.dma_start(w[:], wo[h, :, et * TT:(et + 1) * TT])
                        wo_et.append(w)
                    for j in range(TT // P):
                        t0 = tt * TT + j * P
                        psY = psy.tile([P, TT], f32, tag="psy", name="psY")
                        for h in range(HL):
                            mm("YO", psY[:], onorm[:, h, j * P:(j + 1) * P],
                               wo_et[h][:],
                               start=(h == 0), stop=(h == HL - 1))
                        y_t = yp.tile([P, TT], f32, tag="y")
                        nc.vector.tensor_copy(y_t[:], psY[:])
                        nc.sync.dma_start(y[t0:t0 + P, et * TT:(et + 1) * TT], y_t[:])

                # software pipeline: Q(tt+1) before outproj(tt); outproj(tt-1)
                # e-tile chunks interleave between attention heads of tt so the
                # wo/y DMA bursts spread across the whole t tile
                NET = E // TT
                qt_cur = q_proj(0)
                on_prev = None
                for tt in range(NTT):
                    def after_head(h, _on=on_prev, _tt=tt - 1):
                        if _on is not None:
                            out_proj_chunk(_tt, _on, h)
                    on_cur = attention(tt, qt_cur, after_head)
                    if tt + 1 < NTT:
                        qt_cur = q_proj(tt + 1)
                    on_prev = on_cur
                for et in range(NET):
                    out_proj_chunk(NTT - 1, on_prev, et)
            wqp_cm.__exit__(None, None, None)

    nc.compile()
    return nc


_NC_CACHE = {}


def _get_nc(key=(T, S, E, HL)):
    if key not in _NC_CACHE:
        _NC_CACHE[key] = build_nc(T=key[0], S=key[1], E=key[2], HL=key[3])
    return _NC_CACHE[key]


def kernel(inputs_q, inputs_kv, Wq, Wk, Wv, Wo):
    inputs_q = np.asarray(inputs_q, dtype=np.float32)
    inputs_kv = np.asarray(inputs_kv, dtype=np.float32)
    Wq = np.asarray(Wq, dtype=np.float32)
    Wk = np.asarray(Wk, dtype=np.float32)
    Wv = np.asarray(Wv, dtype=np.float32)
    Wo = np.asarray(Wo, dtype=np.float32)

    nc = _get_nc()

    # shard: core c -> batch c//4, heads [ (c%4)*4, +4 )
    xqT_b = [np.ascontiguousarray(inputs_q[b].T) for b in range(B)]
    xkvT_b = [np.ascontiguousarray(inputs_kv[b].T) for b in range(B)]
    in_maps = []
    for c in range(N_CORES):
        b, g = divmod(c, N_CORES // B)
        h0 = g * HL
        in_maps.append({
            "xqT": xqT_b[b],
            "xkvT": xkvT_b[b],
            "wq": np.ascontiguousarray(Wq[:, h0:h0 + HL, :].reshape(E, HL * D)),
            "wk": np.ascontiguousarray(Wk[:, h0:h0 + HL, :].reshape(E, HL * D)),
            "wv": np.ascontiguousarray(Wv[:, h0:h0 + HL, :].reshape(E, HL * D)),
            "wo": np.ascontiguousarray(Wo[h0:h0 + HL]),
        })

    res = bass_utils.run_bass_kernel_spmd(nc, in_maps, core_ids=list(range(N_CORES)))

    out = np.zeros((B, T, E), dtype=np.float32)
    for c in range(N_CORES):
        b = c // (N_CORES // B)
        out[b] += res.results[c]["y"]
    return out


# revision 16
# speedup vs baseline: 1.3442x; 1.0226x over previous
"""Multi-head dot-product attention (causal, f32) on 8 TRN2 NeuronCores.

Sharding (Megatron-style, per sharding hint): batch (2) x head-groups (4 of
4 heads) = 8 cores. Each core computes q/k/v projections for its 4 heads,
causal attention, and the partial output projection Y_c = sum_h O_h @ Wo_h
for its batch. Host sums the 4 partial Y per batch (the "all-reduce").

Kernel layout strategy: all activations live in transposed [feature, token]
layout so every matmul contracts over the partition dim with N=512 moving
operands:
    QT_h[d,t]  = sum_e Wq[e,hd] * XqT[e,t]      (stationary Wq chunk)
    KT_h[d,s]  = sum_e Wk[e,hd] * XkvT[e,s]
    V[s,hd]    = sum_e XkvT[e,s-blk] * Wv[e,:]  (stationary XkvT chunk)
    LT[s,t]    = KT_h[:,s-blk].T @ QT_h[:,t]    (one matmul, K=D=128)
    P = exp(scale*(LT + causal_bias))           (ACT, writes f32r)
    OT_h[d,t] += V[s-blk,hd].T @ P              (accumulate over s blocks)
    R[1,t]    += ones.T @ P                     (softmax denominator)
    OT_h     *= broadcast(1/R)                  (K=1 outer-product matmul)
    Y[t,e]    = sum_h OT_h[:,t-blk].T @ Wo[h][:,e]
Causal masking skips fully-masked s-blocks and adds -1e10 bias on diagonal
blocks before the exp. Matmuls run in f32r (TF32-class) at full PE rate.

DMA queues: activation tiles + weights split across the two HWDGE queues
(nc.sync / nc.scalar) chunk-interleaved so the first matmul of each phase
starts after ~2 chunk loads; y stores go out on the SWDGE (gpsimd) queue.
"""
import math
import numpy as np

import concourse.bass as bass
import concourse.mybir as mybir
import concourse.tile as tile
from concourse import bacc
from concourse import bass_utils
from concourse.masks import make_identity

f32 = mybir.dt.float32
f32r = mybir.dt.float32r
AF = mybir.ActivationFunctionType

# Problem shape (hardcoded per contract)
B, T, S, E, N, D = 2, 2048, 2048, 2048, 16, 128
N_CORES = 8
HL = 4            # heads per core
P = 128           # partitions


MM_LABELS = {}


def build_nc(T=T, S=S, E=E, HL=HL, TT=512, ST=512):
    """Build the single-core SPMD bass program."""
    NE = E // P           # contraction chunks for projections
    NTT = T // TT         # t tiles
    NST = S // ST         # s tiles in kv phase
    NSB_PER_ST = ST // P  # s blocks per s tile
    NDIAG = TT // P       # diagonal mask patterns
    MBW = TT + (NDIAG - 1) * P  # wide causal-bias pattern
    SCALE = 1.0 / math.sqrt(D)

    nc = bacc.Bacc("TRN2", target_bir_lowering=False, debug=False)

    def mm(label, *args, **kw):
        r = nc.tensor.matmul(*args, **kw)
        MM_LABELS[r.ins.name] = label
        return r

    xqT = nc.dram_tensor("xqT", [E, T], f32r, kind="ExternalInput")
    xkvT = nc.dram_tensor("xkvT", [E, S], f32r, kind="ExternalInput")
    wq = nc.dram_tensor("wq", [E, HL * D], f32r, kind="ExternalInput")
    wk = nc.dram_tensor("wk", [E, HL * D], f32r, kind="ExternalInput")
    wv = nc.dram_tensor("wv", [E, HL * D], f32r, kind="ExternalInput")
    wo = nc.dram_tensor("wo", [HL, D, E], f32r, kind="ExternalInput")
    y = nc.dram_tensor("y", [T, E], f32, kind="ExternalOutput")

    with tile.TileContext(nc) as tc:
        with tc.tile_pool(name="persist", bufs=1) as persist:
            kt_all = persist.tile([P, HL, S], f32r)          # K^T [d, h, s]
            v_all = persist.tile([P, S // P, HL * D], f32r)  # V [s-part, blk, hd]
            mbp = persist.tile([P, MBW], f32r)               # wide causal bias (f32r)
            ident = persist.tile([P, P], f32r)               # f32r identity (mask matmul)
            ones_row = persist.tile([1, P], f32r)            # [1,128] lhsT for K=1 bcast
            ones_col = persist.tile([P, 1], f32r)            # [128,1] lhsT for rowsum

            with tc.tile_pool(name="init", bufs=1) as initp:
                scr = initp.tile([P, MBW], f32)
                nc.gpsimd.memset(scr[:, 0:P], 1.0)
                nc.vector.tensor_copy(ones_row[:], scr[0:1, 0:P])
                nc.vector.tensor_copy(ones_col[:], scr[:, 0:1])
                idf = initp.tile([P, P], f32)
                make_identity(nc, idf[:])
                nc.vector.tensor_copy(ident[:], idf[:])
                # wide pattern W[si, u]: 0 where (u - (NDIAG-1)*P) - si >= 0 else -1e10
                # diag pattern k is the slice mbp[:, (NDIAG-1-k)*P : +TT]
                nc.gpsimd.memset(scr[:], 0.0)
                nc.gpsimd.affine_select(
                    out=scr[:], in_=scr[:],
                    compare_op=mybir.AluOpType.is_ge,
                    fill=-1e10, base=-(NDIAG - 1) * P,
                    pattern=[[1, MBW]], channel_multiplier=-1,
                )
                nc.vector.tensor_copy(mbp[:], scr[:])

            def mb_slice(k):
                off = (NDIAG - 1 - k) * P
                return mbp[:, off:off + TT]

            # wq prefetched during phase B (scalar queue, after wk/wv)
            wqp_cm = tc.tile_pool(name="wqp", bufs=1)
            wqp = wqp_cm.__enter__()
            wq_t = wqp.tile([P, NE, HL * D], f32r)

            # ---------------- Phase B: K^T and V for all heads ----------------
            with (
                tc.tile_pool(name="wkv", bufs=1) as wkvp,
                tc.tile_pool(name="xkv", bufs=6) as xp,
                tc.tile_pool(name="pskt", bufs=HL, space="PSUM") as pskt,
                tc.tile_pool(name="psv", bufs=NSB_PER_ST, space="PSUM") as psv,
            ):
                wk_t = wkvp.tile([P, NE, HL * D], f32r)
                wv_t = wkvp.tile([P, NE, HL * D], f32r)
                # weights on the scalar HWDGE queue, chunk-interleaved
                for e in range(NE):
                    nc.scalar.dma_start(wk_t[:, e, :], wk[e * P:(e + 1) * P, :])
                    nc.scalar.dma_start(wv_t[:, e, :], wv[e * P:(e + 1) * P, :])
                for e in range(NE):
                    nc.scalar.dma_start(wq_t[:, e, :], wq[e * P:(e + 1) * P, :])
                for st in range(NST):
                    psKT = [pskt.tile([P, ST], f32, tag="pskt", name=f"psKT{st}_{h}")
                            for h in range(HL)]
                    psV = [psv.tile([P, HL * D], f32, tag="psv", name=f"psV{st}_{j}")
                           for j in range(NSB_PER_ST)]
                    for e in range(NE):
                        xt = xp.tile([P, ST], f32r, tag="xkv")
                        nc.sync.dma_start(xt[:], xkvT[e * P:(e + 1) * P, st * ST:(st + 1) * ST])
                        for h in range(HL):
                            mm("KT", psKT[h][:], wk_t[:, e, h * D:(h + 1) * D],
                               xt[:], start=(e == 0), stop=(e == NE - 1))
                        for j in range(NSB_PER_ST):
                            mm("V", psV[j][:], xt[:, j * P:(j + 1) * P],
                               wv_t[:, e, :], start=(e == 0), stop=(e == NE - 1))
                    for h in range(HL):
                        nc.vector.tensor_copy(kt_all[:, h, st * ST:(st + 1) * ST], psKT[h][:])
                    for j in range(NSB_PER_ST):
                        nc.vector.tensor_copy(v_all[:, st * NSB_PER_ST + j, :], psV[j][:])

            # ------------- Phase C: Q, attention, output projection -------------
            with (
                tc.tile_pool(name="wop", bufs=8) as wop,
                tc.tile_pool(name="xq", bufs=NE + 2) as xqp,
                tc.tile_pool(name="qt", bufs=2) as qtp,
                tc.tile_pool(name="ep", bufs=4) as epp,
                tc.tile_pool(name="on", bufs=2) as onp,
                tc.tile_pool(name="rb", bufs=1) as rbp,
                tc.tile_pool(name="ysb", bufs=4) as yp,
                tc.tile_pool(name="psq", bufs=2, space="PSUM") as psq,
                tc.tile_pool(name="psy", bufs=2, space="PSUM") as psy,
                tc.tile_pool(name="psl", bufs=2, space="PSUM") as psl,
                tc.tile_pool(name="pso", bufs=1, space="PSUM") as pso,
                tc.tile_pool(name="psr", bufs=1, space="PSUM") as psr,
            ):
                def q_proj(tt):
                    # Q projection: cached x chunks, two sweeps of 2 heads
                    qt_sb = qtp.tile([P, HL, TT], f32r, tag="qt", name=f"qt{tt}")
                    xts = []
                    for e in range(NE):
                        xt = xqp.tile([P, TT], f32r, tag="xq", name=f"xq{tt}_{e}")
                        nc.scalar.dma_start(xt[:], xqT[e * P:(e + 1) * P, tt * TT:(tt + 1) * TT])
                        xts.append(xt)
                    for sweep in range(HL // 2):
                        hs = (sweep * 2, sweep * 2 + 1)
                        psQT = [psq.tile([P, TT], f32, tag="psq", name=f"psQT{tt}_{h}")
                                for h in hs]
                        for e in range(NE):
                            for i, h in enumerate(hs):
                                mm("Q", psQT[i][:], wq_t[:, e, h * D:(h + 1) * D],
                                   xts[e][:], start=(e == 0), stop=(e == NE - 1))
                        for i, h in enumerate(hs):
                            nc.scalar.activation(qt_sb[:, h, :], psQT[i][:], AF.Copy)
                    return qt_sb

                def attention(tt, qt_sb, after_head=None):
                    onorm = onp.tile([P, HL, TT], f32r, tag="on", name=f"on{tt}")
                    nsb = (tt + 1) * (TT // P)  # causal: s blocks 0..nsb-1
                    diag0 = tt * (TT // P)

                    def qk(h, sb):
                        # diagonal blocks: mask bias enters via PE matmul
                        # (identity x pattern), then QK accumulates on top
                        psL = psl.tile([P, TT], f32, tag="psl", name="psL")
                        k = sb - diag0
                        if k >= 0:
                            mm("MB", psL[:], ident[:], mb_slice(k),
                               start=True, stop=False)
                            mm("QK", psL[:], kt_all[:, h, sb * P:(sb + 1) * P],
                               qt_sb[:, h, :], start=False, stop=True)
                        else:
                            mm("QK", psL[:], kt_all[:, h, sb * P:(sb + 1) * P],
                               qt_sb[:, h, :])
                        return psL

                    for h in range(HL):
                        psO = pso.tile([P, TT], f32, tag="pso", name="psO")
                        psR = psr.tile([1, TT], f32, tag="psr", name="psR")
                        psL_next = qk(h, 0)
                        for sb in range(nsb):
                            psL_cur = psL_next
                            ep_t = epp.tile([P, TT], f32r, tag="ep", name="ep")
                            nc.scalar.activation(ep_t[:], psL_cur[:], AF.Exp, scale=SCALE)
                            if sb + 1 < nsb:
                                psL_next = qk(h, sb + 1)  # PE runs ahead of exp
                            mm("AV", psO[:], v_all[:, sb, h * D:(h + 1) * D],
                               ep_t[:], start=(sb == 0), stop=(sb == nsb - 1))
                            mm("R", psR[:], ones_col[:], ep_t[:],
                               start=(sb == 0), stop=(sb == nsb - 1))
                        # normalize: OT_h * broadcast(1/R)
                        rb_f = rbp.tile([P, TT], f32, tag="rbf")
                        nc.vector.reciprocal(rb_f[0:1, :], psR[:])
                        rb_r = rbp.tile([P, TT], f32r, tag="rbr")
                        nc.vector.tensor_copy(rb_r[0:1, :], rb_f[0:1, :])
                        psB = psr.tile([P, TT], f32, tag="psr", name="psB")
                        mm("BC", psB[:], ones_row[:], rb_r[0:1, :])
                        rbc = rbp.tile([P, TT], f32, tag="rbc")
                        nc.scalar.activation(rbc[:], psB[:], AF.Copy)
                        nc.vector.tensor_mul(onorm[:, h, :], psO[:], rbc[:])
                        if after_head is not None:
                            after_head(h)
                    return onorm

                def out_proj_chunk(tt, onorm, et):
                    # one e-tile of the output projection; wo streamed on the
                    # scalar HWDGE queue
                    wo_et = []
                    for h in range(HL):
                        w = wop.tile([P, TT], f32r, tag="wo", name=f"wo{tt}_{et}_{h}")
                        nc.scalar.dma_start(w[:], wo[h, :, et * TT:(et + 1) * TT])
                        wo_et.append(w)
                    for j in range(TT // P):
                        t0 = tt * TT + j * P
                        psY = psy.tile([P, TT], f32, tag="psy", name="psY")
                        for h in range(HL):
                            mm("YO", psY[:], onorm[:, h, j * P:(j + 1) * P],
                               wo_et[h][:],
                               start=(h == 0), stop=(h == HL - 1))
                        y_t = yp.tile([P, TT], f32, tag="y")
                        nc.vector.tensor_copy(y_t[:], psY[:])
                        nc.sync.dma_start(y[t0:t0 + P, et * TT:(et + 1) * TT], y_t[:])

                # software pipeline: Q(tt+1) before outproj(tt); outproj(tt-1)
                # e-tile chunks interleave between attention heads of tt so the
                # wo/y DMA bursts spread across the whole t tile
                NET = E // TT
                qt_cur = q_proj(0)
                on_prev = None
                for tt in range(NTT):
                    def after_head(h, _on=on_prev, _tt=tt - 1):
                        if _on is not None:
                            out_proj_chunk(_tt, _on, h)
                    on_cur = attention(tt, qt_cur, after_head)
                    if tt + 1 < NTT:
                        qt_cur = q_proj(tt + 1)
                    on_prev = on_cur
                for et in range(NET):
                    out_proj_chunk(NTT - 1, on_prev, et)
            wqp_cm.__exit__(None, None, None)

    nc.compile()
    return nc


_NC_CACHE = {}


def _get_nc(key=(T, S, E, HL)):
    if key not in _NC_CACHE:
        _NC_CACHE[key] = build_nc(T=key[0], S=key[1], E=key[2], HL=key[3])
    return _NC_CACHE[key]


def kernel(inputs_q, inputs_kv, Wq, Wk, Wv, Wo):
    inputs_q = np.asarray(inputs_q, dtype=np.float32)
    inputs_kv = np.asarray(inputs_kv, dtype=np.float32)
    Wq = np.asarray(Wq, dtype=np.float32)
    Wk = np.asarray(Wk, dtype=np.float32)
    Wv = np.asarray(Wv, dtype=np.float32)
    Wo = np.asarray(Wo, dtype=np.float32)

    nc = _get_nc()

    # shard: core c -> batch c//4, heads [ (c%4)*4, +4 )
    xqT_b = [np.ascontiguousarray(inputs_q[b].T) for b in range(B)]
    xkvT_b = [np.ascontiguousarray(inputs_kv[b].T) for b in range(B)]
    in_maps = []
    for c in range(N_CORES):
        b, g = divmod(c, N_CORES // B)
        h0 = g * HL
        in_maps.append({
            "xqT": xqT_b[b],
            "xkvT": xkvT_b[b],
            "wq": np.ascontiguousarray(Wq[:, h0:h0 + HL, :].reshape(E, HL * D)),
            "wk": np.ascontiguousarray(Wk[:, h0:h0 + HL, :].reshape(E, HL * D)),
            "wv": np.ascontiguousarray(Wv[:, h0:h0 + HL, :].reshape(E, HL * D)),
            "wo": np.ascontiguousarray(Wo[h0:h0 + HL]),
        })

    res = bass_utils.run_bass_kernel_spmd(nc, in_maps, core_ids=list(range(N_CORES)))

    out = np.zeros((B, T, E), dtype=np.float32)
    for c in range(N_CORES):
        b = c // (N_CORES // B)
        out[b] += res.results[c]["y"]
    return out


# revision 25
# speedup vs baseline: 1.3579x; 1.0102x over previous
"""Multi-head dot-product attention (causal, f32) on 8 TRN2 NeuronCores.

Sharding (Megatron-style, per sharding hint): batch (2) x head-groups (4 of
4 heads) = 8 cores. Each core computes q/k/v projections for its 4 heads,
causal attention, and the partial output projection Y_c = sum_h O_h @ Wo_h
for its batch. Host sums the 4 partial Y per batch (the "all-reduce").

Kernel layout strategy: all activations live in transposed [feature, token]
layout so every matmul contracts over the partition dim with N=512 moving
operands:
    QT_h[d,t]  = sum_e Wq[e,hd] * XqT[e,t]      (stationary Wq chunk)
    KT_h[d,s]  = sum_e Wk[e,hd] * XkvT[e,s]
    V[s,hd]    = sum_e XkvT[e,s-blk] * Wv[e,:]  (stationary XkvT chunk)
    LT[s,t]    = KT_h[:,s-blk].T @ QT_h[:,t]    (one matmul, K=D=128)
    P = exp(scale*(LT + causal_bias))           (ACT, writes f32r)
    OT_h[d,t] += V[s-blk,hd].T @ P              (accumulate over s blocks)
    R[1,t]    += ones.T @ P                     (softmax denominator)
    OT_h     *= broadcast(1/R)                  (K=1 outer-product matmul)
    Y[t,e]    = sum_h OT_h[:,t-blk].T @ Wo[h][:,e]
Causal masking skips fully-masked s-blocks and adds -1e10 bias on diagonal
blocks before the exp. Matmuls run in f32r (TF32-class) at full PE rate.

DMA queues: activation tiles + weights split across the two HWDGE queues
(nc.sync / nc.scalar) chunk-interleaved so the first matmul of each phase
starts after ~2 chunk loads; y stores go out on the SWDGE (gpsimd) queue.
"""
import math
import numpy as np

import concourse.bass as bass
import concourse.mybir as mybir
import concourse.tile as tile
from concourse import bacc
from concourse import bass_utils
from concourse.masks import make_identity

f32 = mybir.dt.float32
f32r = mybir.dt.float32r
AF = mybir.ActivationFunctionType

# Problem shape (hardcoded per contract)
B, T, S, E, N, D = 2, 2048, 2048, 2048, 16, 128
N_CORES = 8
HL = 4            # heads per core
P = 128           # partitions


MM_LABELS = {}


def build_nc(T=T, S=S, E=E, HL=HL, TT=512, ST=512):
    """Build the single-core SPMD bass program."""
    NE = E // P           # contraction chunks for projections
    NTT = T // TT         # t tiles
    NST = S // ST         # s tiles in kv phase
    NSB_PER_ST = ST // P  # s blocks per s tile
    NDIAG = TT // P       # diagonal mask patterns
    MBW = TT + (NDIAG - 1) * P  # wide causal-bias pattern
    SCALE = 1.0 / math.sqrt(D)

    nc = bacc.Bacc("TRN2", target_bir_lowering=False, debug=False)

    def mm(label, *args, **kw):
        r = nc.tensor.matmul(*args, **kw)
        MM_LABELS[r.ins.name] = label
        return r

    xqT = nc.dram_tensor("xqT", [E, T], f32r, kind="ExternalInput")
    xkvT = nc.dram_tensor("xkvT", [E, S], f32r, kind="ExternalInput")
    wq = nc.dram_tensor("wq", [E, HL * D], f32r, kind="ExternalInput")
    wk = nc.dram_tensor("wk", [E, HL * D], f32r, kind="ExternalInput")
    wv = nc.dram_tensor("wv", [E, HL * D], f32r, kind="ExternalInput")
    wo = nc.dram_tensor("wo", [HL, D, E], f32r, kind="ExternalInput")
    y = nc.dram_tensor("y", [T, E], f32, kind="ExternalOutput")

    with tile.TileContext(nc) as tc:
        with tc.tile_pool(name="persist", bufs=1) as persist:
            kt_all = persist.tile([P, HL, S], f32r)          # K^T [d, h, s]
            v_all = persist.tile([P, S // P, HL * D], f32r)  # V [s-part, blk, hd]
            mbp = persist.tile([P, MBW], f32r)               # wide causal bias (f32r)
            ident = persist.tile([P, P], f32r)               # f32r identity (mask matmul)
            ones_row = persist.tile([1, P], f32r)            # [1,128] lhsT for K=1 bcast
            ones_col = persist.tile([P, 1], f32r)            # [128,1] lhsT for rowsum

            with tc.tile_pool(name="init", bufs=1) as initp:
                scr = initp.tile([P, MBW], f32)
                nc.gpsimd.memset(scr[:, 0:P], 1.0)
                nc.vector.tensor_copy(ones_row[:], scr[0:1, 0:P])
                nc.vector.tensor_copy(ones_col[:], scr[:, 0:1])
                idf = initp.tile([P, P], f32)
                make_identity(nc, idf[:])
                nc.vector.tensor_copy(ident[:], idf[:])
                # wide pattern W[si, u]: 0 where (u - (NDIAG-1)*P) - si >= 0 else -1e10
                # diag pattern k is the slice mbp[:, (NDIAG-1-k)*P : +TT]
                nc.gpsimd.memset(scr[:], 0.0)
                nc.gpsimd.affine_select(
                    out=scr[:], in_=scr[:],
                    compare_op=mybir.AluOpType.is_ge,
                    fill=-1e10, base=-(NDIAG - 1) * P,
                    pattern=[[1, MBW]], channel_multiplier=-1,
                )
                nc.vector.tensor_copy(mbp[:], scr[:])

            def mb_slice(k):
                off = (NDIAG - 1 - k) * P
                return mbp[:, off:off + TT]

            # wq prefetched during phase B (scalar queue, after wk/wv)
            wqp_cm = tc.tile_pool(name="wqp", bufs=1)
            wqp = wqp_cm.__enter__()
            wq_t = wqp.tile([P, NE, HL * D], f32r)

            # ---------------- Phase B: K^T and V for all heads ----------------
            with (
                tc.tile_pool(name="wkv", bufs=1) as wkvp,
                tc.tile_pool(name="xkv", bufs=6) as xp,
                tc.tile_pool(name="pskt", bufs=HL, space="PSUM") as pskt,
                tc.tile_pool(name="psv", bufs=NSB_PER_ST, space="PSUM") as psv,
            ):
                wk_t = wkvp.tile([P, NE, HL * D], f32r)
                wv_t = wkvp.tile([P, NE, HL * D], f32r)
                # weights on the scalar HWDGE queue, chunk-interleaved
                for e in range(NE):
                    nc.scalar.dma_start(wk_t[:, e, :], wk[e * P:(e + 1) * P, :])
                    nc.scalar.dma_start(wv_t[:, e, :], wv[e * P:(e + 1) * P, :])
                for e in range(NE):
                    nc.scalar.dma_start(wq_t[:, e, :], wq[e * P:(e + 1) * P, :])
                for st in range(NST):
                    psKT = [pskt.tile([P, ST], f32, tag="pskt", name=f"psKT{st}_{h}")
                            for h in range(HL)]
                    psV = [psv.tile([P, HL * D], f32, tag="psv", name=f"psV{st}_{j}")
                           for j in range(NSB_PER_ST)]
                    for e in range(NE):
                        xt = xp.tile([P, ST], f32r, tag="xkv")
                        nc.sync.dma_start(xt[:], xkvT[e * P:(e + 1) * P, st * ST:(st + 1) * ST])
                        for h in range(HL):
                            mm("KT", psKT[h][:], wk_t[:, e, h * D:(h + 1) * D],
                               xt[:], start=(e == 0), stop=(e == NE - 1))
                        for j in range(NSB_PER_ST):
                            mm("V", psV[j][:], xt[:, j * P:(j + 1) * P],
                               wv_t[:, e, :], start=(e == 0), stop=(e == NE - 1))
                    for h in range(HL):
                        nc.vector.tensor_copy(kt_all[:, h, st * ST:(st + 1) * ST], psKT[h][:])
                    for j in range(NSB_PER_ST):
                        nc.vector.tensor_copy(v_all[:, st * NSB_PER_ST + j, :], psV[j][:])

            # ------------- Phase C: Q, attention, output projection -------------
            with (
                tc.tile_pool(name="wop", bufs=7) as wop,
                tc.tile_pool(name="xq", bufs=NE + 2) as xqp,
                tc.tile_pool(name="qt", bufs=2) as qtp,
                tc.tile_pool(name="ep", bufs=4) as epp,
                tc.tile_pool(name="on", bufs=2) as onp,
                tc.tile_pool(name="rb", bufs=1) as rbp,
                tc.tile_pool(name="es", bufs=1) as esp,
                tc.tile_pool(name="ysb", bufs=3) as yp,
                tc.tile_pool(name="psq", bufs=2, space="PSUM") as psq,
                tc.tile_pool(name="psy", bufs=2, space="PSUM") as psy,
                tc.tile_pool(name="psl", bufs=2, space="PSUM") as psl,
                tc.tile_pool(name="pso", bufs=1, space="PSUM") as pso,
                tc.tile_pool(name="psr", bufs=1, space="PSUM") as psr,
            ):
                def q_proj(tt):
                    # Q projection: cached x chunks, two sweeps of 2 heads
                    qt_sb = qtp.tile([P, HL, TT], f32r, tag="qt", name=f"qt{tt}")
                    xts = []
                    for e in range(NE):
                        xt = xqp.tile([P, TT], f32r, tag="xq", name=f"xq{tt}_{e}")
                        nc.scalar.dma_start(xt[:], xqT[e * P:(e + 1) * P, tt * TT:(tt + 1) * TT])
                        xts.append(xt)
                    for sweep in range(HL // 2):
                        hs = (sweep * 2, sweep * 2 + 1)
                        psQT = [psq.tile([P, TT], f32, tag="psq", name=f"psQT{tt}_{h}")
                                for h in hs]
                        for e in range(NE):
                            for i, h in enumerate(hs):
                                mm("Q", psQT[i][:], wq_t[:, e, h * D:(h + 1) * D],
                                   xts[e][:], start=(e == 0), stop=(e == NE - 1))
                        for i, h in enumerate(hs):
                            nc.scalar.activation(qt_sb[:, h, :], psQT[i][:], AF.Copy)
                    return qt_sb

                def attention(tt, qt_sb, after_head=None):
                    onorm = onp.tile([P, HL, TT], f32r, tag="on", name=f"on{tt}")
                    nsb = (tt + 1) * (TT // P)  # causal: s blocks 0..nsb-1
                    diag0 = tt * (TT // P)

                    def qk(h, sb):
                        # diagonal blocks: mask bias enters via PE matmul
                        # (identity x pattern), then QK accumulates on top
                        psL = psl.tile([P, TT], f32, tag="psl", name="psL")
                        k = sb - diag0
                        if k >= 0:
                            mm("MB", psL[:], ident[:], mb_slice(k),
                               start=True, stop=False)
                            mm("QK", psL[:], kt_all[:, h, sb * P:(sb + 1) * P],
                               qt_sb[:, h, :], start=False, stop=True)
                        else:
                            mm("QK", psL[:], kt_all[:, h, sb * P:(sb + 1) * P],
                               qt_sb[:, h, :])
                        return psL

                    for h in range(HL):
                        psO = pso.tile([P, TT], f32, tag="pso", name="psO")
                        # block-wise exp sums accumulate on DVE (off the PE
                        # critical path); one ones-matmul at the end reduces
                        # the partition dim for the softmax denominator
                        esum = esp.tile([P, TT], f32, tag="es", name="esum")
                        psL_next = qk(h, 0)
                        for sb in range(nsb):
                            psL_cur = psL_next
                            ep_t = epp.tile([P, TT], f32r, tag="ep", name="ep")
                            nc.scalar.activation(ep_t[:], psL_cur[:], AF.Exp, scale=SCALE)
                            if sb + 1 < nsb:
                                psL_next = qk(h, sb + 1)  # PE runs ahead of exp
                            mm("AV", psO[:], v_all[:, sb, h * D:(h + 1) * D],
                               ep_t[:], start=(sb == 0), stop=(sb == nsb - 1))
                            if sb == 0:
                                nc.vector.tensor_copy(esum[:], ep_t[:].bitcast(f32))
                            else:
                                nc.vector.tensor_add(esum[:], esum[:], ep_t[:].bitcast(f32))
                        esum_r = esp.tile([P, TT], f32r, tag="esr", name="esum_r")
                        nc.vector.tensor_copy(esum_r[:], esum[:])
                        psR = psr.tile([1, TT], f32, tag="psr", name="psR")
                        mm("R", psR[:], ones_col[:], esum_r[:])
                        # normalize: OT_h * broadcast(1/R)
                        rb_f = rbp.tile([P, TT], f32, tag="rbf")
                        nc.vector.reciprocal(rb_f[0:1, :], psR[:])
                        rb_r = rbp.tile([P, TT], f32r, tag="rbr")
                        nc.vector.tensor_copy(rb_r[0:1, :], rb_f[0:1, :])
                        psB = psr.tile([P, TT], f32, tag="psr", name="psB")
                        mm("BC", psB[:], ones_row[:], rb_r[0:1, :])
                        rbc = rbp.tile([P, TT], f32, tag="rbc")
                        nc.scalar.activation(rbc[:], psB[:], AF.Copy)
                        nc.vector.tensor_mul(onorm[:, h, :], psO[:], rbc[:])
                        if after_head is not None:
                            after_head(h)
                    return onorm

                def out_proj_chunk(tt, onorm, et):
                    # one e-tile of the output projection; wo streamed on the
                    # scalar HWDGE queue
                    wo_et = []
                    for h in range(HL):
                        w = wop.tile([P, TT], f32r, tag="wo", name=f"wo{tt}_{et}_{h}")
                        nc.scalar.dma_start(w[:], wo[h, :, et * TT:(et + 1) * TT])
                        wo_et.append(w)
                    for j in range(TT // P):
                        t0 = tt * TT + j * P
                        psY = psy.tile([P, TT], f32, tag="psy", name="psY")
                        for h in range(HL):
                            mm("YO", psY[:], onorm[:, h, j * P:(j + 1) * P],
                               wo_et[h][:],
                               start=(h == 0), stop=(h == HL - 1))
                        y_t = yp.tile([P, TT], f32, tag="y")
                        nc.vector.tensor_copy(y_t[:], psY[:])
                        nc.sync.dma_start(y[t0:t0 + P, et * TT:(et + 1) * TT], y_t[:])

                # software pipeline: Q(tt+1) before outproj(tt); outproj(tt-1)
                # e-tile chunks interleave between attention heads of tt so the
                # wo/y DMA bursts spread across the whole t tile
                NET = E // TT
                qt_cur = q_proj(0)
                on_prev = None
                for tt in range(NTT):
                    def after_head(h, _on=on_prev, _tt=tt - 1):
                        if _on is not None:
                            out_proj_chunk(_tt, _on, h)
                    on_cur = attention(tt, qt_cur, after_head)
                    if tt + 1 < NTT:
                        qt_cur = q_proj(tt + 1)
                    on_prev = on_cur
                for et in range(NET):
                    out_proj_chunk(NTT - 1, on_prev, et)
            wqp_cm.__exit__(None, None, None)

    nc.compile()
    return nc


_NC_CACHE = {}


def _get_nc(key=(T, S, E, HL)):
    if key not in _NC_CACHE:
        _NC_CACHE[key] = build_nc(T=key[0], S=key[1], E=key[2], HL=key[3])
    return _NC_CACHE[key]


def kernel(inputs_q, inputs_kv, Wq, Wk, Wv, Wo):
    inputs_q = np.asarray(inputs_q, dtype=np.float32)
    inputs_kv = np.asarray(inputs_kv, dtype=np.float32)
    Wq = np.asarray(Wq, dtype=np.float32)
    Wk = np.asarray(Wk, dtype=np.float32)
    Wv = np.asarray(Wv, dtype=np.float32)
    Wo = np.asarray(Wo, dtype=np.float32)

    nc = _get_nc()

    # shard: core c -> batch c//4, heads [ (c%4)*4, +4 )
    xqT_b = [np.ascontiguousarray(inputs_q[b].T) for b in range(B)]
    xkvT_b = [np.ascontiguousarray(inputs_kv[b].T) for b in range(B)]
    in_maps = []
    for c in range(N_CORES):
        b, g = divmod(c, N_CORES // B)
        h0 = g * HL
        in_maps.append({
            "xqT": xqT_b[b],
            "xkvT": xkvT_b[b],
            "wq": np.ascontiguousarray(Wq[:, h0:h0 + HL, :].reshape(E, HL * D)),
            "wk": np.ascontiguousarray(Wk[:, h0:h0 + HL, :].reshape(E, HL * D)),
            "wv": np.ascontiguousarray(Wv[:, h0:h0 + HL, :].reshape(E, HL * D)),
            "wo": np.ascontiguousarray(Wo[h0:h0 + HL]),
        })

    res = bass_utils.run_bass_kernel_spmd(nc, in_maps, core_ids=list(range(N_CORES)))

    out = np.zeros((B, T, E), dtype=np.float32)
    for c in range(N_CORES):
        b = c // (N_CORES // B)
        out[b] += res.results[c]["y"]
    return out


# revision 31
# speedup vs baseline: 1.3737x; 1.0116x over previous
"""Multi-head dot-product attention (causal, f32) on 8 TRN2 NeuronCores.

Sharding (Megatron-style, per sharding hint): batch (2) x head-groups (4 of
4 heads) = 8 cores. Each core computes q/k/v projections for its 4 heads,
causal attention, and the partial output projection Y_c = sum_h O_h @ Wo_h
for its batch. Host sums the 4 partial Y per batch (the "all-reduce").

Kernel layout strategy: all activations live in transposed [feature, token]
layout so every matmul contracts over the partition dim with N=512 moving
operands:
    QT_h[d,t]  = sum_e Wq[e,hd] * XqT[e,t]      (stationary Wq chunk)
    KT_h[d,s]  = sum_e Wk[e,hd] * XkvT[e,s]
    V[s,hd]    = sum_e XkvT[e,s-blk] * Wv[e,:]  (stationary XkvT chunk)
    LT[s,t]    = KT_h[:,s-blk].T @ QT_h[:,t]    (one matmul, K=D=128)
    P = exp(scale*(LT + causal_bias))           (ACT, writes f32r)
    OT_h[d,t] += V[s-blk,hd].T @ P              (accumulate over s blocks)
    R[1,t]    += ones.T @ P                     (softmax denominator)
    OT_h     *= broadcast(1/R)                  (K=1 outer-product matmul)
    Y[t,e]    = sum_h OT_h[:,t-blk].T @ Wo[h][:,e]
Causal masking skips fully-masked s-blocks and adds -1e10 bias on diagonal
blocks before the exp. Matmuls run in f32r (TF32-class) at full PE rate.

DMA queues: activation tiles + weights split across the two HWDGE queues
(nc.sync / nc.scalar) chunk-interleaved so the first matmul of each phase
starts after ~2 chunk loads; y stores go out on the SWDGE (gpsimd) queue.
"""
import math
import numpy as np

import concourse.bass as bass
import concourse.mybir as mybir
import concourse.tile as tile
from concourse import bacc
from concourse import bass_utils
from concourse.masks import make_identity

f32 = mybir.dt.float32
f32r = mybir.dt.float32r
AF = mybir.ActivationFunctionType

# Problem shape (hardcoded per contract)
B, T, S, E, N, D = 2, 2048, 2048, 2048, 16, 128
N_CORES = 8
HL = 4            # heads per core
P = 128           # partitions


MM_LABELS = {}


def build_nc(T=T, S=S, E=E, HL=HL, TT=512, ST=512):
    """Build the single-core SPMD bass program."""
    NE = E // P           # contraction chunks for projections
    NTT = T // TT         # t tiles
    NST = S // ST         # s tiles in kv phase
    NSB_PER_ST = ST // P  # s blocks per s tile
    NDIAG = TT // P       # diagonal mask patterns
    MBW = TT + (NDIAG - 1) * P  # wide causal-bias pattern
    SCALE = 1.0 / math.sqrt(D)

    nc = bacc.Bacc("TRN2", target_bir_lowering=False, debug=False)

    def mm(label, *args, **kw):
        r = nc.tensor.matmul(*args, **kw)
        MM_LABELS[r.ins.name] = label
        return r

    xqT = nc.dram_tensor("xqT", [E, T], f32r, kind="ExternalInput")
    xkvT = nc.dram_tensor("xkvT", [E, S], f32r, kind="ExternalInput")
    wq = nc.dram_tensor("wq", [E, HL * D], f32r, kind="ExternalInput")
    wk = nc.dram_tensor("wk", [E, HL * D], f32r, kind="ExternalInput")
    wv = nc.dram_tensor("wv", [E, HL * D], f32r, kind="ExternalInput")
    wo = nc.dram_tensor("wo", [HL, D, E], f32r, kind="ExternalInput")
    y = nc.dram_tensor("y", [T, E], f32, kind="ExternalOutput")

    with tile.TileContext(nc) as tc:
        with tc.tile_pool(name="persist", bufs=1) as persist:
            kt_all = persist.tile([P, HL, S], f32r)          # K^T [d, h, s]
            v_all = persist.tile([P, S // P, HL * D], f32r)  # V [s-part, blk, hd]
            mbp = persist.tile([P, MBW], f32r)               # wide causal bias (f32r)
            ident = persist.tile([P, P], f32r)               # f32r identity (mask matmul)
            ones_row = persist.tile([1, P], f32r)            # [1,128] lhsT for K=1 bcast
            ones_col = persist.tile([P, 1], f32r)            # [128,1] lhsT for rowsum

            with tc.tile_pool(name="init", bufs=1) as initp:
                scr = initp.tile([P, MBW], f32)
                nc.gpsimd.memset(scr[:, 0:P], 1.0)
                nc.vector.tensor_copy(ones_row[:], scr[0:1, 0:P])
                nc.vector.tensor_copy(ones_col[:], scr[:, 0:1])
                idf = initp.tile([P, P], f32)
                make_identity(nc, idf[:])
                nc.vector.tensor_copy(ident[:], idf[:])
                # wide pattern W[si, u]: 0 where (u - (NDIAG-1)*P) - si >= 0 else -1e10
                # diag pattern k is the slice mbp[:, (NDIAG-1-k)*P : +TT]
                nc.gpsimd.memset(scr[:], 0.0)
                nc.gpsimd.affine_select(
                    out=scr[:], in_=scr[:],
                    compare_op=mybir.AluOpType.is_ge,
                    fill=-1e10, base=-(NDIAG - 1) * P,
                    pattern=[[1, MBW]], channel_multiplier=-1,
                )
                nc.vector.tensor_copy(mbp[:], scr[:])

            def mb_slice(k):
                off = (NDIAG - 1 - k) * P
                return mbp[:, off:off + TT]

            # wq prefetched during phase B (scalar queue, after wk/wv)
            wqp_cm = tc.tile_pool(name="wqp", bufs=1)
            wqp = wqp_cm.__enter__()
            wq_t = wqp.tile([P, NE, HL * D], f32r)

            # ---------------- Phase B: K^T and V for all heads ----------------
            with (
                tc.tile_pool(name="wkv", bufs=1) as wkvp,
                tc.tile_pool(name="xkv", bufs=6) as xp,
                tc.tile_pool(name="pskt", bufs=HL, space="PSUM") as pskt,
                tc.tile_pool(name="psv", bufs=NSB_PER_ST, space="PSUM") as psv,
            ):
                wk_t = wkvp.tile([P, NE, HL * D], f32r)
                wv_t = wkvp.tile([P, NE, HL * D], f32r)
                # weights on the scalar HWDGE queue, chunk-interleaved
                for e in range(NE):
                    nc.scalar.dma_start(wk_t[:, e, :], wk[e * P:(e + 1) * P, :])
                    nc.scalar.dma_start(wv_t[:, e, :], wv[e * P:(e + 1) * P, :])
                for e in range(NE):
                    nc.scalar.dma_start(wq_t[:, e, :], wq[e * P:(e + 1) * P, :])
                for st in range(NST):
                    psKT = [pskt.tile([P, ST], f32, tag="pskt", name=f"psKT{st}_{h}")
                            for h in range(HL)]
                    psV = [psv.tile([P, HL * D], f32, tag="psv", name=f"psV{st}_{j}")
                           for j in range(NSB_PER_ST)]
                    for e in range(NE):
                        xt = xp.tile([P, ST], f32r, tag="xkv")
                        nc.sync.dma_start(xt[:], xkvT[e * P:(e + 1) * P, st * ST:(st + 1) * ST])
                        for h in range(HL):
                            mm("KT", psKT[h][:], wk_t[:, e, h * D:(h + 1) * D],
                               xt[:], start=(e == 0), stop=(e == NE - 1))
                        for j in range(NSB_PER_ST):
                            mm("V", psV[j][:], xt[:, j * P:(j + 1) * P],
                               wv_t[:, e, :], start=(e == 0), stop=(e == NE - 1))
                    for h in range(HL):
                        nc.vector.tensor_copy(kt_all[:, h, st * ST:(st + 1) * ST], psKT[h][:])
                    for j in range(NSB_PER_ST):
                        nc.vector.tensor_copy(v_all[:, st * NSB_PER_ST + j, :], psV[j][:])

            # ------------- Phase C: Q, attention, output projection -------------
            with (
                tc.tile_pool(name="wop", bufs=6) as wop,
                tc.tile_pool(name="xq", bufs=NE + 1) as xqp,
                tc.tile_pool(name="qt", bufs=2) as qtp,
                tc.tile_pool(name="ep", bufs=4) as epp,
                tc.tile_pool(name="on", bufs=2) as onp,
                tc.tile_pool(name="rb", bufs=1) as rbp,
                tc.tile_pool(name="es", bufs=1) as esp,
                tc.tile_pool(name="os", bufs=2) as osp,
                tc.tile_pool(name="ysb", bufs=3) as yp,
                tc.tile_pool(name="psq", bufs=2, space="PSUM") as psq,
                tc.tile_pool(name="psy", bufs=2, space="PSUM") as psy,
                tc.tile_pool(name="psl", bufs=2, space="PSUM") as psl,
                tc.tile_pool(name="pso", bufs=1, space="PSUM") as pso,
                tc.tile_pool(name="psr", bufs=1, space="PSUM") as psr,
            ):
                def q_proj(tt):
                    # Q projection: cached x chunks, two sweeps of 2 heads
                    qt_sb = qtp.tile([P, HL, TT], f32r, tag="qt", name=f"qt{tt}")
                    xts = []
                    for e in range(NE):
                        xt = xqp.tile([P, TT], f32r, tag="xq", name=f"xq{tt}_{e}")
                        nc.scalar.dma_start(xt[:], xqT[e * P:(e + 1) * P, tt * TT:(tt + 1) * TT])
                        xts.append(xt)
                    for sweep in range(HL // 2):
                        hs = (sweep * 2, sweep * 2 + 1)
                        psQT = [psq.tile([P, TT], f32, tag="psq", name=f"psQT{tt}_{h}")
                                for h in hs]
                        for e in range(NE):
                            for i, h in enumerate(hs):
                                mm("Q", psQT[i][:], wq_t[:, e, h * D:(h + 1) * D],
                                   xts[e][:], start=(e == 0), stop=(e == NE - 1))
                        for i, h in enumerate(hs):
                            nc.scalar.activation(qt_sb[:, h, :], psQT[i][:], AF.Copy)
                    return qt_sb

                def attention(tt, qt_sb, after_head=None):
                    onorm = onp.tile([P, HL, TT], f32r, tag="on", name=f"on{tt}")
                    nsb = (tt + 1) * (TT // P)  # causal: s blocks 0..nsb-1
                    diag0 = tt * (TT // P)

                    def qk(h, sb):
                        # diagonal blocks: mask bias enters via PE matmul
                        # (identity x pattern), then QK accumulates on top
                        psL = psl.tile([P, TT], f32, tag="psl", name="psL")
                        k = sb - diag0
                        if k >= 0:
                            mm("MB", psL[:], ident[:], mb_slice(k),
                               start=True, stop=False)
                            mm("QK", psL[:], kt_all[:, h, sb * P:(sb + 1) * P],
                               qt_sb[:, h, :], start=False, stop=True)
                        else:
                            mm("QK", psL[:], kt_all[:, h, sb * P:(sb + 1) * P],
                               qt_sb[:, h, :])
                        return psL

                    for h in range(HL):
                        psO = pso.tile([P, TT], f32, tag="pso", name="psO")
                        # block-wise exp sums accumulate on DVE (off the PE
                        # critical path); one ones-matmul at the end reduces
                        # the partition dim for the softmax denominator
                        esum = esp.tile([P, TT], f32, tag="es", name="esum")
                        psL_next = qk(h, 0)
                        for sb in range(nsb):
                            psL_cur = psL_next
                            ep_t = epp.tile([P, TT], f32r, tag="ep", name="ep")
                            nc.scalar.activation(ep_t[:], psL_cur[:], AF.Exp, scale=SCALE)
                            if sb + 1 < nsb:
                                psL_next = qk(h, sb + 1)  # PE runs ahead of exp
                            mm("AV", psO[:], v_all[:, sb, h * D:(h + 1) * D],
                               ep_t[:], start=(sb == 0), stop=(sb == nsb - 1))
                            if sb == 0:
                                nc.vector.tensor_copy(esum[:], ep_t[:].bitcast(f32))
                            else:
                                nc.vector.tensor_add(esum[:], esum[:], ep_t[:].bitcast(f32))
                        # free the psO bank ASAP so the next head's AV can
                        # start while this head's normalization chain drains
                        osum = osp.tile([P, TT], f32, tag="os", name="osum")
                        nc.vector.tensor_copy(osum[:], psO[:])
                        esum_r = esp.tile([P, TT], f32r, tag="esr", name="esum_r")
                        nc.vector.tensor_copy(esum_r[:], esum[:])
                        psR = psr.tile([1, TT], f32, tag="psr", name="psR")
                        mm("R", psR[:], ones_col[:], esum_r[:])
                        # normalize: OT_h * broadcast(1/R); 1/R rounds to f32r
                        # on the copy anyway, so write it f32r directly
                        rb_r = rbp.tile([P, TT], f32r, tag="rbr")
                        with nc.allow_low_precision(reason="1/R feeds an f32r matmul"):
                            nc.vector.reciprocal(rb_r[0:1, :], psR[:])
                        psB = psr.tile([P, TT], f32, tag="psr", name="psB")
                        mm("BC", psB[:], ones_row[:], rb_r[0:1, :])
                        nc.vector.tensor_mul(onorm[:, h, :], psB[:], osum[:])
                        if after_head is not None:
                            after_head(h)
                    return onorm

                def out_proj_chunk(tt, onorm, et):
                    # one e-tile of the output projection; wo streamed on the
                    # scalar HWDGE queue
                    wo_et = []
                    for h in range(HL):
                        w = wop.tile([P, TT], f32r, tag="wo", name=f"wo{tt}_{et}_{h}")
                        nc.scalar.dma_start(w[:], wo[h, :, et * TT:(et + 1) * TT])
                        wo_et.append(w)
                    for j in range(TT // P):
                        t0 = tt * TT + j * P
                        psY = psy.tile([P, TT], f32, tag="psy", name="psY")
                        for h in range(HL):
                            mm("YO", psY[:], onorm[:, h, j * P:(j + 1) * P],
                               wo_et[h][:],
                               start=(h == 0), stop=(h == HL - 1))
                        y_t = yp.tile([P, TT], f32, tag="y")
                        nc.vector.tensor_copy(y_t[:], psY[:])
                        nc.sync.dma_start(y[t0:t0 + P, et * TT:(et + 1) * TT], y_t[:])

                # software pipeline: Q(tt+1) before outproj(tt); outproj(tt-1)
                # e-tile chunks interleave between attention heads of tt so the
                # wo/y DMA bursts spread across the whole t tile
                NET = E // TT
                qt_cur = q_proj(0)
                on_prev = None
                for tt in range(NTT):
                    def after_head(h, _on=on_prev, _tt=tt - 1):
                        if _on is not None:
                            out_proj_chunk(_tt, _on, h)
                    on_cur = attention(tt, qt_cur, after_head)
                    if tt + 1 < NTT:
                        qt_cur = q_proj(tt + 1)
                    on_prev = on_cur
                for et in range(NET):
                    out_proj_chunk(NTT - 1, on_prev, et)
            wqp_cm.__exit__(None, None, None)

    nc.compile()
    return nc


_NC_CACHE = {}


def _get_nc(key=(T, S, E, HL)):
    if key not in _NC_CACHE:
        _NC_CACHE[key] = build_nc(T=key[0], S=key[1], E=key[2], HL=key[3])
    return _NC_CACHE[key]


def kernel(inputs_q, inputs_kv, Wq, Wk, Wv, Wo):
    inputs_q = np.asarray(inputs_q, dtype=np.float32)
    inputs_kv = np.asarray(inputs_kv, dtype=np.float32)
    Wq = np.asarray(Wq, dtype=np.float32)
    Wk = np.asarray(Wk, dtype=np.float32)
    Wv = np.asarray(Wv, dtype=np.float32)
    Wo = np.asarray(Wo, dtype=np.float32)

    nc = _get_nc()

    # shard: core c -> batch c//4, heads [ (c%4)*4, +4 )
    xqT_b = [np.ascontiguousarray(inputs_q[b].T) for b in range(B)]
    xkvT_b = [np.ascontiguousarray(inputs_kv[b].T) for b in range(B)]
    in_maps = []
    for c in range(N_CORES):
        b, g = divmod(c, N_CORES // B)
        h0 = g * HL
        in_maps.append({
            "xqT": xqT_b[b],
            "xkvT": xkvT_b[b],
            "wq": np.ascontiguousarray(Wq[:, h0:h0 + HL, :].reshape(E, HL * D)),
            "wk": np.ascontiguousarray(Wk[:, h0:h0 + HL, :].reshape(E, HL * D)),
            "wv": np.ascontiguousarray(Wv[:, h0:h0 + HL, :].reshape(E, HL * D)),
            "wo": np.ascontiguousarray(Wo[h0:h0 + HL]),
        })

    res = bass_utils.run_bass_kernel_spmd(nc, in_maps, core_ids=list(range(N_CORES)))

    out = np.zeros((B, T, E), dtype=np.float32)
    for c in range(N_CORES):
        b = c // (N_CORES // B)
        out[b] += res.results[c]["y"]
    return out
